# revision 13
# baseline (speedup 1.0000x reference)
"""AttentionBlock (GroupNorm + 8-head self-attention + out-proj + residual) on 8 trn2 cores.

Sharding: core = (batch b, query-half ih).  Each core gets x[b] rolled so that
"its" 1024 query positions are columns 0:1024; K/V are computed over the full
(rolled) L=2048, which is sound because attention and the group-norm statistics
are invariant to a permutation of key/value positions.  Output is the core's
[512, 1024] slice of proj + residual; the host reassembles [4, 512, 2048].

On-device layout highlights:
 - group norm folded to per-channel (x - s1) * s2 via bn_stats + PE-transpose
   stat aggregation; applied in-place on x tiles (residual saved first).
 - qkv / proj matmuls run as float32r (full-rate fp32 PE mode).
 - attention computes S^T = K^T Q per head (softmax dim on partitions), exp on
   ACT (scale=1/8 folded in, no max-subtraction: |S/8| <~ 6 so fp32/bf16 safe),
   O = V^T-augmented @ exp in bf16 with a ones column producing the softmax
   denominator on PSUM partition 64 for free.
 - two heads are packed per S^T chunk via PE row tiling (K=64 each).
"""

import sys

sys.path.insert(0, "/opt/trn_rl_repo")

import numpy as np
import ml_dtypes

import concourse.bass as bass
import concourse.mybir as mybir
import concourse.tile as tile
from concourse import bacc
from concourse.vector_clock import ScopedClock, VectorClock
from concourse.bass_utils import run_bass_kernel_spmd

F32 = mybir.dt.float32
F32R = mybir.dt.float32r
BF16 = mybir.dt.bfloat16
AX = mybir.AxisListType
OP = mybir.AluOpType
ACTF = mybir.ActivationFunctionType

B, C, L = 4, 512, 2048
H, D = 8, 64
G, EPS = 32, 1e-5
LQ = L // 2          # queries per core
CT = C // 128        # channel tiles
NJC = L // 128       # key chunks of 128
NIB = LQ // 512      # 512-wide query blocks


class _SplitDrainTC(tile.TileContext):
    """Stock exit puts every outstanding proc's wait on one SP Drain; this
    walrus build caps sync-waits per instruction, so spread them over
    single-wait NOPs first."""

    def _drain_and_barrier(self, tick_clock, wait_clock):
        g = tick_clock.global_clock
        for proc in range(len(g)):
            if g[proc] == 0:
                continue
            vc = VectorClock([0] * len(g))
            vc.require_at_least(proc, g[proc])
            nop = self.nc.sync.nop(hint=f"split_drain_{proc}")
            wait_clock.add_sem_waits(nop.ins, ScopedClock({None: vc}))
        self.nc.sync.drain()
        self.nc.all_engine_barrier()
        assert self.sems is not None
        popped = self.nc._tile_sem_poison_stack.pop()
        assert popped is self._sem_poison
        self.nc.clear_and_free_semaphores(list(self.sems.allocated().values()))
        self.nc.all_engine_barrier()


def _r(ap):
    return ap.bitcast(F32R)


def build_nc(reps: int = 1):
    nc = bacc.Bacc("TRN2", target_bir_lowering=False, num_devices=8)

    xd = nc.declare_dram_parameter("x", [C, L], F32, isOutput=False)
    wqkvT = nc.declare_dram_parameter("wqkvT", [C, 3 * C], BF16, isOutput=False)
    woutT = nc.declare_dram_parameter("woutT", [C, C], BF16, isOutput=False)
    gnwd = nc.declare_dram_parameter("gnw", [CT, 128], F32, isOutput=False)
    gnbd = nc.declare_dram_parameter("gnb", [CT, 128], F32, isOutput=False)
    boutd = nc.declare_dram_parameter("bout", [128, CT], F32, isOutput=False)
    indd = nc.declare_dram_parameter("ind", [H, C], BF16, isOutput=False)
    identd = nc.declare_dram_parameter("ident", [128, 128], F32, isOutput=False)
    yd = nc.declare_dram_parameter("y", [C, LQ], F32, isOutput=True)

    import contextlib

    with _SplitDrainTC(nc) as tc:
        with (
            tc.For_i(0, reps, 1) if reps > 1 else contextlib.nullcontext()
        ), tc.tile_pool(name="persist", bufs=1) as pp:
            x_sb = [pp.tile([128, L], F32, name=f"x{t}", tag=f"x{t}") for t in range(CT)]
            wq_sb = [pp.tile([128, 3 * C], BF16, name=f"wq{t}", tag=f"wq{t}") for t in range(CT)]
            wo_sb = [pp.tile([128, C], BF16, name=f"wo{t}", tag=f"wo{t}") for t in range(CT)]
            q_sb = [pp.tile([128, LQ], BF16, name=f"q{t}", tag=f"q{t}") for t in range(CT)]
            k_sb = [pp.tile([128, L], BF16, name=f"k{t}", tag=f"k{t}") for t in range(CT)]
            vt_sb = [pp.tile([128, H * (D + 1)], BF16, name=f"vt{t}", tag=f"vt{t}") for t in range(NJC)]
            oh_sb = [pp.tile([128, LQ], F32, name=f"oh{t}", tag=f"oh{t}") for t in range(CT)]
            nx_sb = [pp.tile([128, L], BF16, name=f"nx{t}", tag=f"nx{t}") for t in range(CT)]
            ohb_sb = [pp.tile([128, LQ], BF16, name=f"ohb{t}", tag=f"ohb{t}") for t in range(CT)]
            rcp_sb = [pp.tile([H, 512], BF16, name=f"rcp{i}", tag=f"rcp{i}") for i in range(NIB)]
            gnw_sb = pp.tile([CT, 128], F32, name="gnw", tag="gnw")
            gnb_sb = pp.tile([CT, 128], F32, name="gnb", tag="gnb")
            bout_sb = pp.tile([128, CT], F32, name="bout", tag="bout")
            ind_sb = pp.tile([H, C], BF16, name="ind", tag="ind")
            ident_sb = pp.tile([128, 128], F32, name="ident", tag="ident")
            sparam_sb = pp.tile([128, 2, CT], F32, name="sparam", tag="sparam")
            denom_sb = [pp.tile([H, 512], F32, name=f"dn{i}", tag=f"dn{i}") for i in range(NIB)]

            for t in range(CT):
                nc.sync.dma_start(x_sb[t][:], xd[128 * t : 128 * t + 128, :])
                nc.sync.dma_start(wq_sb[t][:], wqkvT[128 * t : 128 * t + 128, :])
                nc.sync.dma_start(wo_sb[t][:], woutT[128 * t : 128 * t + 128, :])
            nc.sync.dma_start(gnw_sb[:], gnwd[:])
            nc.sync.dma_start(gnb_sb[:], gnbd[:])
            nc.sync.dma_start(bout_sb[:], boutd[:])
            nc.sync.dma_start(ind_sb[:], indd[:])
            nc.sync.dma_start(ident_sb[:], identd[:])

            # ---------------- group norm statistics ----------------
            with (
                tc.tile_pool(name="gtmp", bufs=2) as gp,
                tc.tile_pool(name="gps", bufs=2, space="PSUM") as gpp,
            ):
                # stats_all col t = channel-mean(tile t), col 32+t = channel-var:
                # after PE transpose, means land on partitions 0..3 and vars on
                # 32..35 (engine APs may only start at partition 0/32/64/96).
                stats_all = gp.tile([128, 36], F32, name="stats_all", tag="stats_all")
                nc.vector.memset(stats_all[:], 0.0)
                for t in range(CT):
                    st6 = gp.tile([128, 4, 6], F32, name="st6", tag="st6")
                    for sg in range(4):
                        nc.vector.bn_stats(
                            out=st6[:, sg, :],
                            in_=x_sb[t][:, 512 * sg : 512 * sg + 512],
                        )
                    sa = stats_all[:]
                    mv_out = bass.AP(
                        tensor=sa.tensor, offset=sa.offset + t, ap=[sa.ap[0], [32, 2]]
                    )
                    nc.vector.bn_aggr(out=mv_out, in_=st6[:])

                st_ps = gpp.tile([36, 128], F32, name="st_ps", tag="st_ps")
                nc.tensor.transpose(st_ps[:], stats_all[:], ident_sb[:])
                statsT = gp.tile([36, 128], F32, name="statsT", tag="statsT")
                nc.vector.tensor_copy(statsT[:], st_ps[:])

                mred = gp.tile([4, 8], F32, name="mred", tag="mred")
                nc.vector.tensor_reduce(
                    out=mred[:],
                    in_=statsT[0:4, :].rearrange("p (g s) -> p g s", s=16),
                    axis=AX.X,
                    op=OP.add,
                )
                vred = gp.tile([4, 8], F32, name="vred", tag="vred")
                nc.vector.tensor_reduce(
                    out=vred[:],
                    in_=statsT[32:36, :].rearrange("p (g s) -> p g s", s=16),
                    axis=AX.X,
                    op=OP.add,
                )
                sq = gp.tile([4, 128], F32, name="sq", tag="sq")
                nc.vector.tensor_mul(sq[:], statsT[0:4, :], statsT[0:4, :])
                sqred = gp.tile([4, 8], F32, name="sqred", tag="sqred")
                nc.vector.tensor_reduce(
                    out=sqred[:],
                    in_=sq[:].rearrange("p (g s) -> p g s", s=16),
                    axis=AX.X,
                    op=OP.add,
                )
                mg = gp.tile([4, 8], F32, name="mg", tag="mg")
                nc.vector.tensor_scalar_mul(mg[:], mred[:], 1.0 / 16)
                # vg = red_var/16 + sqred/16 - mg^2
                vg = gp.tile([4, 8], F32, name="vg", tag="vg")
                nc.vector.tensor_scalar_mul(vg[:], vred[:], 1.0 / 16)
                nc.vector.scalar_tensor_tensor(
                    out=vg[:],
                    in0=sqred[:],
                    scalar=1.0 / 16,
                    in1=vg[:],
                    op0=OP.mult,
                    op1=OP.add,
                )
                mg2 = gp.tile([4, 8], F32, name="mg2", tag="mg2")
                nc.vector.tensor_mul(mg2[:], mg[:], mg[:])
                nc.vector.tensor_sub(vg[:], vg[:], mg2[:])
                # rstd = 1/sqrt(vg + eps)
                epst = gp.tile([4, 1], F32, name="epst", tag="epst")
                nc.vector.memset(epst[:], EPS)
                nc.scalar.activation(out=vg[:], in_=vg[:], func=ACTF.Sqrt, bias=epst[:])
                nc.vector.reciprocal(out=vg[:], in_=vg[:])

                # broadcast group -> channels: [4, 8] -> [4, 128]
                def bcast16(src):
                    a = src.ap
                    return bass.AP(
                        tensor=src.tensor, offset=src.offset, ap=[a[0], a[1], [0, 16]]
                    )

                rstd_bc = gp.tile([4, 128], F32, name="rstd_bc", tag="rstd_bc")
                nc.vector.tensor_copy(
                    rstd_bc[:].rearrange("p (g s) -> p g s", s=16), bcast16(vg[:])
                )
                mg_bc = gp.tile([4, 128], F32, name="mg_bc", tag="mg_bc")
                nc.vector.tensor_copy(
                    mg_bc[:].rearrange("p (g s) -> p g s", s=16), bcast16(mg[:])
                )
                s2 = gp.tile([4, 128], F32, name="s2", tag="s2")
                nc.vector.tensor_mul(s2[:], rstd_bc[:], gnw_sb[0:4, :])
                s1 = gp.tile([4, 128], F32, name="s1", tag="s1")
                nc.vector.reciprocal(out=s1[:], in_=s2[:])
                nc.vector.tensor_mul(s1[:], s1[:], gnb_sb[0:4, :])
                nc.vector.tensor_sub(s1[:], mg_bc[:], s1[:])

                sp_ps = gpp.tile([128, 2, CT], F32, name="sp_ps", tag="sp_ps")
                nc.tensor.transpose(sp_ps[:, 0, :], s1[:], ident_sb[0:4, 0:4])
                nc.tensor.transpose(sp_ps[:, 1, :], s2[:], ident_sb[0:4, 0:4])
                nc.vector.tensor_copy(sparam_sb[:], sp_ps[:])

            # group-norm apply: nx = (x - s1) * s2, cast to bf16
            for t in range(CT):
                nc.vector.tensor_scalar(
                    out=nx_sb[t][:],
                    in0=x_sb[t][:],
                    scalar1=sparam_sb[:, 0, t : t + 1],
                    scalar2=sparam_sb[:, 1, t : t + 1],
                    op0=OP.subtract,
                    op1=OP.mult,
                )

            # ---------------- qkv ----------------
            with tc.tile_pool(name="psqkv", bufs=6, space="PSUM") as pq:
                for t in range(CT):  # q: only first LQ columns
                    for nb in range(LQ // 512):
                        ps = pq.tile([128, 512], F32, name="qkv", tag="qkv")
                        for c in range(CT):
                            nc.tensor.matmul(
                                ps[:],
                                wq_sb[c][:, 128 * t : 128 * t + 128],
                                nx_sb[c][:, 512 * nb : 512 * nb + 512],
                                start=(c == 0),
                                stop=(c == CT - 1),
                            )
                        nc.vector.tensor_copy(q_sb[t][:, 512 * nb : 512 * nb + 512], ps[:])
                for t in range(CT):  # k: full L
                    for nb in range(L // 512):
                        ps = pq.tile([128, 512], F32, name="qkv", tag="qkv")
                        for c in range(CT):
                            nc.tensor.matmul(
                                ps[:],
                                wq_sb[c][:, C + 128 * t : C + 128 * t + 128],
                                nx_sb[c][:, 512 * nb : 512 * nb + 512],
                                start=(c == 0),
                                stop=(c == CT - 1),
                            )
                        nc.vector.tensor_copy(k_sb[t][:, 512 * nb : 512 * nb + 512], ps[:])
                for lt in range(NJC):  # v^T: [l, hd] via nx as stationary
                    ps = pq.tile([128, 512], F32, name="qkv", tag="qkv")
                    for c in range(CT):
                        nc.tensor.matmul(
                            ps[:],
                            nx_sb[c][:, 128 * lt : 128 * lt + 128],
                            wq_sb[c][:, 2 * C : 3 * C],
                            start=(c == 0),
                            stop=(c == CT - 1),
                        )
                    vt_view = vt_sb[lt][:].rearrange("p (h c) -> p h c", c=D + 1)
                    nc.vector.memset(vt_view[:, :, D : D + 1], 1.0)
                    nc.vector.tensor_copy(
                        vt_view[:, :, 0:D],
                        ps[:].rearrange("p (h c) -> p h c", c=D),
                    )

            # ---------------- attention ----------------
            with (
                tc.tile_pool(name="psS", bufs=1, space="PSUM") as pS,
                tc.tile_pool(name="psO", bufs=1, space="PSUM") as pO,
                tc.tile_pool(name="expp", bufs=3) as ep,
                tc.tile_pool(name="stg", bufs=2) as stp,
            ):
                for pair in range(H // 2):
                    kt, qt = k_sb[pair], q_sb[pair]
                    for ib in range(NIB):
                        ops = pO.tile([128, 2, 512], F32, name="O", tag="O")
                        slots = pS.tile([128, 6, 512], F32, name="S", tag="S")
                        qA = qt[0:64, 512 * ib : 512 * ib + 512]
                        qB = qt[64:128, 512 * ib : 512 * ib + 512]
                        for jc in range(NJC):
                            s = (2 * jc) % 6
                            nc.tensor.matmul(
                                slots[:, s, :],
                                kt[0:64, 128 * jc : 128 * jc + 128],
                                qA,
                                start=True,
                                stop=True,
                            )
                            nc.tensor.matmul(
                                slots[:, s + 1, :],
                                kt[64:128, 128 * jc : 128 * jc + 128],
                                qB,
                                start=True,
                                stop=True,
                            )
                            et = ep.tile([128, 1024], BF16, name="exp", tag="exp")
                            nc.scalar.activation(
                                out=et[:],
                                in_=slots[:, s : s + 2, :],
                                func=ACTF.Exp,
                                scale=float(D) ** -0.5,
                            )
                            for h01 in range(2):
                                hcol = (D + 1) * (2 * pair + h01)
                                nc.tensor.matmul(
                                    ops[0:65, h01, :],
                                    vt_sb[jc][:, hcol : hcol + D + 1],
                                    et[:, 512 * h01 : 512 * h01 + 512],
                                    start=(jc == 0),
                                    stop=(jc == NJC - 1),
                                )
                        for h01 in range(2):
                            h = 2 * pair + h01
                            nc.vector.tensor_copy(
                                oh_sb[h // 2][
                                    64 * (h % 2) : 64 * (h % 2) + 64,
                                    512 * ib : 512 * ib + 512,
                                ],
                                ops[0:64, h01, :],
                            )
                            stg = stp.tile([128, 512], F32, name="stg", tag="stg")
                            nc.vector.tensor_copy(stg[64:65, :], ops[64:65, h01, :])
                            nc.sync.dma_start(
                                denom_sb[ib][h : h + 1, :], stg[64:65, :]
                            )

            # ---------------- normalize + projection + residual ----------------
            with tc.tile_pool(name="pspr", bufs=4, space="PSUM") as pr:
                for ib in range(NIB):
                    with nc.allow_low_precision(reason="bf16 softmax recip intended"):
                        nc.vector.reciprocal(out=rcp_sb[ib][:], in_=denom_sb[ib][:])
                    for t in range(CT):
                        bps = pr.tile([128, 512], F32, name="pr", tag="pr")
                        nc.tensor.matmul(
                            bps[:],
                            ind_sb[0:H, 128 * t : 128 * t + 128],
                            rcp_sb[ib][:],
                            start=True,
                            stop=True,
                        )
                        sl = slice(512 * ib, 512 * ib + 512)
                        nc.vector.tensor_mul(ohb_sb[t][:, sl], oh_sb[t][:, sl], bps[:])
                for t in range(CT):
                    for nb in range(LQ // 512):
                        ps = pr.tile([128, 512], F32, name="pr", tag="pr")
                        for c in range(CT):
                            nc.tensor.matmul(
                                ps[:],
                                wo_sb[c][:, 128 * t : 128 * t + 128],
                                ohb_sb[c][:, 512 * nb : 512 * nb + 512],
                                start=(c == 0),
                                stop=(c == CT - 1),
                            )
                        sl = slice(512 * nb, 512 * nb + 512)
                        # y = (proj + b_out) + residual, fused
                        nc.vector.scalar_tensor_tensor(
                            out=oh_sb[t][:, sl],
                            in0=ps[:],
                            scalar=bout_sb[:, t : t + 1],
                            in1=x_sb[t][:, sl],
                            op0=OP.add,
                            op1=OP.add,
                        )
                for t in range(CT):
                    nc.sync.dma_start(yd[128 * t : 128 * t + 128, :], oh_sb[t][:])

    nc.compile()
    return nc


_NC_CACHE = None


def _get_nc():
    global _NC_CACHE
    if _NC_CACHE is None:
        _NC_CACHE = build_nc()
    return _NC_CACHE


def _host_inputs(x, gn_w, gn_b, w_qkv, w_out, b_out):
    w_qkvT = np.ascontiguousarray(w_qkv.T).astype(ml_dtypes.bfloat16)
    w_outT = np.ascontiguousarray(w_out.T).astype(ml_dtypes.bfloat16)
    ind = np.zeros((H, C), ml_dtypes.bfloat16)
    for h in range(H):
        for t in range(CT):
            for m in range(128):
                if h == 2 * t + (m >= 64):
                    ind[h, 128 * t + m] = 1.0
    ident = np.eye(128, dtype=np.float32)
    shared = {
        "wqkvT": w_qkvT,
        "woutT": w_outT,
        "gnw": np.ascontiguousarray(gn_w.reshape(CT, 128), np.float32),
        "gnb": np.ascontiguousarray(gn_b.reshape(CT, 128), np.float32),
        "bout": np.ascontiguousarray(b_out.reshape(CT, 128).T, np.float32),
        "ind": ind,
        "ident": ident,
    }
    in_maps = []
    for core in range(8):
        b, ih = core // 2, core % 2
        xb = np.asarray(x[b], np.float32)
        if ih:
            xb = np.concatenate([xb[:, LQ:], xb[:, :LQ]], axis=1)
        in_maps.append({"x": np.ascontiguousarray(xb), **shared})
    return in_maps


def kernel(x, gn_w, gn_b, w_qkv, w_out, b_out):
    nc = _get_nc()
    in_maps = _host_inputs(
        np.asarray(x), np.asarray(gn_w), np.asarray(gn_b),
        np.asarray(w_qkv), np.asarray(w_out), np.asarray(b_out),
    )
    res = run_bass_kernel_spmd(nc, in_maps, list(range(8)))
    y = np.empty((B, C, L), np.float32)
    for core in range(8):
        b, ih = core // 2, core % 2
        y[b][:, ih * LQ : (ih + 1) * LQ] = res.results[core]["y"]
    return y


# revision 14
# speedup vs baseline: 1.4062x; 1.4062x over previous
"""AttentionBlock (GroupNorm + 8-head self-attention + out-proj + residual) on 8 trn2 cores.

Sharding: core = (batch b, query-half ih).  Each core gets x[b] rolled so that
"its" 1024 query positions are columns 0:1024; K/V are computed over the full
(rolled) L=2048, which is sound because attention and the group-norm statistics
are invariant to a permutation of key/value positions.  Output is the core's
[512, 1024] slice of proj + residual; the host reassembles [4, 512, 2048].

On-device layout highlights:
 - group norm folded to per-channel (x - s1) * s2 via bn_stats + PE-transpose
   stat aggregation; applied in-place on x tiles (residual saved first).
 - qkv / proj matmuls run as float32r (full-rate fp32 PE mode).
 - attention computes S^T = K^T Q per head (softmax dim on partitions), exp on
   ACT (scale=1/8 folded in, no max-subtraction: |S/8| <~ 6 so fp32/bf16 safe),
   O = V^T-augmented @ exp in bf16 with a ones column producing the softmax
   denominator on PSUM partition 64 for free.
 - two heads are packed per S^T chunk via PE row tiling (K=64 each).
"""

import sys

sys.path.insert(0, "/opt/trn_rl_repo")

import numpy as np
import ml_dtypes

import concourse.bass as bass
import concourse.mybir as mybir
import concourse.tile as tile
from concourse import bacc
from concourse.vector_clock import ScopedClock, VectorClock
from concourse.bass_utils import run_bass_kernel_spmd

F32 = mybir.dt.float32
F32R = mybir.dt.float32r
BF16 = mybir.dt.bfloat16
AX = mybir.AxisListType
OP = mybir.AluOpType
ACTF = mybir.ActivationFunctionType

B, C, L = 4, 512, 2048
H, D = 8, 64
G, EPS = 32, 1e-5
LQ = L // 2          # queries per core
CT = C // 128        # channel tiles
NJC = L // 128       # key chunks of 128
NIB = LQ // 512      # 512-wide query blocks


class _SplitDrainTC(tile.TileContext):
    """Stock exit puts every outstanding proc's wait on one SP Drain; this
    walrus build caps sync-waits per instruction, so spread them over
    single-wait NOPs first."""

    def _drain_and_barrier(self, tick_clock, wait_clock):
        g = tick_clock.global_clock
        for proc in range(len(g)):
            if g[proc] == 0:
                continue
            vc = VectorClock([0] * len(g))
            vc.require_at_least(proc, g[proc])
            nop = self.nc.sync.nop(hint=f"split_drain_{proc}")
            wait_clock.add_sem_waits(nop.ins, ScopedClock({None: vc}))
        self.nc.sync.drain()
        self.nc.all_engine_barrier()
        assert self.sems is not None
        popped = self.nc._tile_sem_poison_stack.pop()
        assert popped is self._sem_poison
        self.nc.clear_and_free_semaphores(list(self.sems.allocated().values()))
        self.nc.all_engine_barrier()


def _r(ap):
    return ap.bitcast(F32R)


def build_nc(reps: int = 1):
    nc = bacc.Bacc("TRN2", target_bir_lowering=False, num_devices=8)

    xd = nc.declare_dram_parameter("x", [C, L], F32, isOutput=False)
    wqkvT = nc.declare_dram_parameter("wqkvT", [C, 3 * C], BF16, isOutput=False)
    woutT = nc.declare_dram_parameter("woutT", [C, C], BF16, isOutput=False)
    gnwd = nc.declare_dram_parameter("gnw", [CT, 128], F32, isOutput=False)
    gnbd = nc.declare_dram_parameter("gnb", [CT, 128], F32, isOutput=False)
    boutd = nc.declare_dram_parameter("bout", [128, CT], F32, isOutput=False)
    indd = nc.declare_dram_parameter("ind", [H, C], BF16, isOutput=False)
    identd = nc.declare_dram_parameter("ident", [128, 128], F32, isOutput=False)
    yd = nc.declare_dram_parameter("y", [C, LQ], F32, isOutput=True)

    import contextlib

    with _SplitDrainTC(nc) as tc:
        with (
            tc.For_i(0, reps, 1) if reps > 1 else contextlib.nullcontext()
        ), tc.tile_pool(name="persist", bufs=1) as pp:
            x_sb = [pp.tile([128, L], F32, name=f"x{t}", tag=f"x{t}") for t in range(CT)]
            wq_sb = [pp.tile([128, 3 * C], BF16, name=f"wq{t}", tag=f"wq{t}") for t in range(CT)]
            wo_sb = [pp.tile([128, C], BF16, name=f"wo{t}", tag=f"wo{t}") for t in range(CT)]
            q_sb = [pp.tile([128, LQ], BF16, name=f"q{t}", tag=f"q{t}") for t in range(CT)]
            k_sb = [pp.tile([128, L], BF16, name=f"k{t}", tag=f"k{t}") for t in range(CT)]
            vt_sb = [pp.tile([128, H * (D + 1)], BF16, name=f"vt{t}", tag=f"vt{t}") for t in range(NJC)]
            oh_sb = [pp.tile([128, LQ], F32, name=f"oh{t}", tag=f"oh{t}") for t in range(CT)]
            nx_sb = [pp.tile([128, L], BF16, name=f"nx{t}", tag=f"nx{t}") for t in range(CT)]
            ohb_sb = [pp.tile([128, LQ], BF16, name=f"ohb{t}", tag=f"ohb{t}") for t in range(CT)]
            rcp_sb = [pp.tile([H, 512], BF16, name=f"rcp{i}", tag=f"rcp{i}") for i in range(NIB)]
            gnw_sb = pp.tile([CT, 128], F32, name="gnw", tag="gnw")
            gnb_sb = pp.tile([CT, 128], F32, name="gnb", tag="gnb")
            bout_sb = pp.tile([128, CT], F32, name="bout", tag="bout")
            ind_sb = pp.tile([H, C], BF16, name="ind", tag="ind")
            ident_sb = pp.tile([128, 128], F32, name="ident", tag="ident")
            sparam_sb = pp.tile([128, 2, CT], F32, name="sparam", tag="sparam")
            denom_sb = [pp.tile([H, 512], F32, name=f"dn{i}", tag=f"dn{i}") for i in range(NIB)]

            # spread big loads across both HWDGE engines (SP, ACT) plus the
            # SWDGE path so they don't serialize on one queue set, and chunk x
            # so bn_stats can start on the first 512 columns early.
            dma_engs = [nc.sync, nc.scalar, nc.gpsimd]
            di = 0
            for t in range(CT):
                for sg in range(4):
                    csl = slice(512 * sg, 512 * sg + 512)
                    dma_engs[di % 3].dma_start(
                        x_sb[t][:, csl], xd[128 * t : 128 * t + 128, csl]
                    )
                    di += 1
            for t in range(CT):
                for half in range(2):
                    wsl = slice(768 * half, 768 * half + 768)
                    dma_engs[di % 3].dma_start(
                        wq_sb[t][:, wsl], wqkvT[128 * t : 128 * t + 128, wsl]
                    )
                    di += 1
                dma_engs[di % 3].dma_start(
                    wo_sb[t][:], woutT[128 * t : 128 * t + 128, :]
                )
                di += 1
            nc.sync.dma_start(gnw_sb[:], gnwd[:])
            nc.sync.dma_start(gnb_sb[:], gnbd[:])
            nc.sync.dma_start(bout_sb[:], boutd[:])
            nc.sync.dma_start(ind_sb[:], indd[:])
            nc.sync.dma_start(ident_sb[:], identd[:])

            # ---------------- group norm statistics ----------------
            with (
                tc.tile_pool(name="gtmp", bufs=2) as gp,
                tc.tile_pool(name="gps", bufs=2, space="PSUM") as gpp,
            ):
                # stats_all col t = channel-mean(tile t), col 32+t = channel-var:
                # after PE transpose, means land on partitions 0..3 and vars on
                # 32..35 (engine APs may only start at partition 0/32/64/96).
                stats_all = gp.tile([128, 36], F32, name="stats_all", tag="stats_all")
                nc.vector.memset(stats_all[:], 0.0)
                for t in range(CT):
                    st6 = gp.tile([128, 4, 6], F32, name="st6", tag="st6")
                    for sg in range(4):
                        nc.vector.bn_stats(
                            out=st6[:, sg, :],
                            in_=x_sb[t][:, 512 * sg : 512 * sg + 512],
                        )
                    sa = stats_all[:]
                    mv_out = bass.AP(
                        tensor=sa.tensor, offset=sa.offset + t, ap=[sa.ap[0], [32, 2]]
                    )
                    nc.vector.bn_aggr(out=mv_out, in_=st6[:])

                st_ps = gpp.tile([36, 128], F32, name="st_ps", tag="st_ps")
                nc.tensor.transpose(st_ps[:], stats_all[:], ident_sb[:])
                statsT = gp.tile([36, 128], F32, name="statsT", tag="statsT")
                nc.vector.tensor_copy(statsT[:], st_ps[:])

                mred = gp.tile([4, 8], F32, name="mred", tag="mred")
                nc.vector.tensor_reduce(
                    out=mred[:],
                    in_=statsT[0:4, :].rearrange("p (g s) -> p g s", s=16),
                    axis=AX.X,
                    op=OP.add,
                )
                vred = gp.tile([4, 8], F32, name="vred", tag="vred")
                nc.vector.tensor_reduce(
                    out=vred[:],
                    in_=statsT[32:36, :].rearrange("p (g s) -> p g s", s=16),
                    axis=AX.X,
                    op=OP.add,
                )
                sq = gp.tile([4, 128], F32, name="sq", tag="sq")
                nc.vector.tensor_mul(sq[:], statsT[0:4, :], statsT[0:4, :])
                sqred = gp.tile([4, 8], F32, name="sqred", tag="sqred")
                nc.vector.tensor_reduce(
                    out=sqred[:],
                    in_=sq[:].rearrange("p (g s) -> p g s", s=16),
                    axis=AX.X,
                    op=OP.add,
                )
                mg = gp.tile([4, 8], F32, name="mg", tag="mg")
                nc.vector.tensor_scalar_mul(mg[:], mred[:], 1.0 / 16)
                # vg = red_var/16 + sqred/16 - mg^2
                vg = gp.tile([4, 8], F32, name="vg", tag="vg")
                nc.vector.tensor_scalar_mul(vg[:], vred[:], 1.0 / 16)
                nc.vector.scalar_tensor_tensor(
                    out=vg[:],
                    in0=sqred[:],
                    scalar=1.0 / 16,
                    in1=vg[:],
                    op0=OP.mult,
                    op1=OP.add,
                )
                mg2 = gp.tile([4, 8], F32, name="mg2", tag="mg2")
                nc.vector.tensor_mul(mg2[:], mg[:], mg[:])
                nc.vector.tensor_sub(vg[:], vg[:], mg2[:])
                # rstd = 1/sqrt(vg + eps)
                epst = gp.tile([4, 1], F32, name="epst", tag="epst")
                nc.vector.memset(epst[:], EPS)
                nc.scalar.activation(out=vg[:], in_=vg[:], func=ACTF.Sqrt, bias=epst[:])
                nc.vector.reciprocal(out=vg[:], in_=vg[:])

                # broadcast group -> channels: [4, 8] -> [4, 128]
                def bcast16(src):
                    a = src.ap
                    return bass.AP(
                        tensor=src.tensor, offset=src.offset, ap=[a[0], a[1], [0, 16]]
                    )

                rstd_bc = gp.tile([4, 128], F32, name="rstd_bc", tag="rstd_bc")
                nc.vector.tensor_copy(
                    rstd_bc[:].rearrange("p (g s) -> p g s", s=16), bcast16(vg[:])
                )
                mg_bc = gp.tile([4, 128], F32, name="mg_bc", tag="mg_bc")
                nc.vector.tensor_copy(
                    mg_bc[:].rearrange("p (g s) -> p g s", s=16), bcast16(mg[:])
                )
                s2 = gp.tile([4, 128], F32, name="s2", tag="s2")
                nc.vector.tensor_mul(s2[:], rstd_bc[:], gnw_sb[0:4, :])
                s1 = gp.tile([4, 128], F32, name="s1", tag="s1")
                nc.vector.reciprocal(out=s1[:], in_=s2[:])
                nc.vector.tensor_mul(s1[:], s1[:], gnb_sb[0:4, :])
                nc.vector.tensor_sub(s1[:], mg_bc[:], s1[:])

                sp_ps = gpp.tile([128, 2, CT], F32, name="sp_ps", tag="sp_ps")
                nc.tensor.transpose(sp_ps[:, 0, :], s1[:], ident_sb[0:4, 0:4])
                nc.tensor.transpose(sp_ps[:, 1, :], s2[:], ident_sb[0:4, 0:4])
                nc.vector.tensor_copy(sparam_sb[:], sp_ps[:])

            # group-norm apply: nx = (x - s1) * s2, cast to bf16
            for t in range(CT):
                nc.vector.tensor_scalar(
                    out=nx_sb[t][:],
                    in0=x_sb[t][:],
                    scalar1=sparam_sb[:, 0, t : t + 1],
                    scalar2=sparam_sb[:, 1, t : t + 1],
                    op0=OP.subtract,
                    op1=OP.mult,
                )

            # ---------------- qkv ----------------
            with tc.tile_pool(name="psqkv", bufs=6, space="PSUM") as pq:
                for t in range(CT):  # q: only first LQ columns
                    for nb in range(LQ // 512):
                        ps = pq.tile([128, 512], F32, name="qkv", tag="qkv")
                        for c in range(CT):
                            nc.tensor.matmul(
                                ps[:],
                                wq_sb[c][:, 128 * t : 128 * t + 128],
                                nx_sb[c][:, 512 * nb : 512 * nb + 512],
                                start=(c == 0),
                                stop=(c == CT - 1),
                            )
                        nc.vector.tensor_copy(q_sb[t][:, 512 * nb : 512 * nb + 512], ps[:])
                for t in range(CT):  # k: full L
                    for nb in range(L // 512):
                        ps = pq.tile([128, 512], F32, name="qkv", tag="qkv")
                        for c in range(CT):
                            nc.tensor.matmul(
                                ps[:],
                                wq_sb[c][:, C + 128 * t : C + 128 * t + 128],
                                nx_sb[c][:, 512 * nb : 512 * nb + 512],
                                start=(c == 0),
                                stop=(c == CT - 1),
                            )
                        nc.vector.tensor_copy(k_sb[t][:, 512 * nb : 512 * nb + 512], ps[:])
                for lt in range(NJC):  # v^T: [l, hd] via nx as stationary
                    ps = pq.tile([128, 512], F32, name="qkv", tag="qkv")
                    for c in range(CT):
                        nc.tensor.matmul(
                            ps[:],
                            nx_sb[c][:, 128 * lt : 128 * lt + 128],
                            wq_sb[c][:, 2 * C : 3 * C],
                            start=(c == 0),
                            stop=(c == CT - 1),
                        )
                    vt_view = vt_sb[lt][:].rearrange("p (h c) -> p h c", c=D + 1)
                    nc.vector.memset(vt_view[:, :, D : D + 1], 1.0)
                    nc.vector.tensor_copy(
                        vt_view[:, :, 0:D],
                        ps[:].rearrange("p (h c) -> p h c", c=D),
                    )

            # ---------------- attention ----------------
            with (
                tc.tile_pool(name="psS", bufs=1, space="PSUM") as pS,
                tc.tile_pool(name="psO", bufs=1, space="PSUM") as pO,
                tc.tile_pool(name="expp", bufs=3) as ep,
                tc.tile_pool(name="stg", bufs=2) as stp,
            ):
                for pair in range(H // 2):
                    kt, qt = k_sb[pair], q_sb[pair]
                    for ib in range(NIB):
                        ops = pO.tile([128, 2, 512], F32, name="O", tag="O")
                        slots = pS.tile([128, 6, 512], F32, name="S", tag="S")
                        qA = qt[0:64, 512 * ib : 512 * ib + 512]
                        qB = qt[64:128, 512 * ib : 512 * ib + 512]
                        for jc in range(NJC):
                            s = (2 * jc) % 6
                            nc.tensor.matmul(
                                slots[:, s, :],
                                kt[0:64, 128 * jc : 128 * jc + 128],
                                qA,
                                start=True,
                                stop=True,
                            )
                            nc.tensor.matmul(
                                slots[:, s + 1, :],
                                kt[64:128, 128 * jc : 128 * jc + 128],
                                qB,
                                start=True,
                                stop=True,
                            )
                            et = ep.tile([128, 1024], BF16, name="exp", tag="exp")
                            nc.scalar.activation(
                                out=et[:],
                                in_=slots[:, s : s + 2, :],
                                func=ACTF.Exp,
                                scale=float(D) ** -0.5,
                            )
                            for h01 in range(2):
                                hcol = (D + 1) * (2 * pair + h01)
                                nc.tensor.matmul(
                                    ops[0:65, h01, :],
                                    vt_sb[jc][:, hcol : hcol + D + 1],
                                    et[:, 512 * h01 : 512 * h01 + 512],
                                    start=(jc == 0),
                                    stop=(jc == NJC - 1),
                                )
                        for h01 in range(2):
                            h = 2 * pair + h01
                            nc.vector.tensor_copy(
                                oh_sb[h // 2][
                                    64 * (h % 2) : 64 * (h % 2) + 64,
                                    512 * ib : 512 * ib + 512,
                                ],
                                ops[0:64, h01, :],
                            )
                            stg = stp.tile([128, 512], F32, name="stg", tag="stg")
                            nc.vector.tensor_copy(stg[64:65, :], ops[64:65, h01, :])
                            nc.sync.dma_start(
                                denom_sb[ib][h : h + 1, :], stg[64:65, :]
                            )

            # ---------------- normalize + projection + residual ----------------
            with tc.tile_pool(name="pspr", bufs=4, space="PSUM") as pr:
                for ib in range(NIB):
                    with nc.allow_low_precision(reason="bf16 softmax recip intended"):
                        nc.vector.reciprocal(out=rcp_sb[ib][:], in_=denom_sb[ib][:])
                    for t in range(CT):
                        bps = pr.tile([128, 512], F32, name="pr", tag="pr")
                        nc.tensor.matmul(
                            bps[:],
                            ind_sb[0:H, 128 * t : 128 * t + 128],
                            rcp_sb[ib][:],
                            start=True,
                            stop=True,
                        )
                        sl = slice(512 * ib, 512 * ib + 512)
                        nc.vector.tensor_mul(ohb_sb[t][:, sl], oh_sb[t][:, sl], bps[:])
                for t in range(CT):
                    for nb in range(LQ // 512):
                        ps = pr.tile([128, 512], F32, name="pr", tag="pr")
                        for c in range(CT):
                            nc.tensor.matmul(
                                ps[:],
                                wo_sb[c][:, 128 * t : 128 * t + 128],
                                ohb_sb[c][:, 512 * nb : 512 * nb + 512],
                                start=(c == 0),
                                stop=(c == CT - 1),
                            )
                        sl = slice(512 * nb, 512 * nb + 512)
                        # y = (proj + b_out) + residual, fused
                        nc.vector.scalar_tensor_tensor(
                            out=oh_sb[t][:, sl],
                            in0=ps[:],
                            scalar=bout_sb[:, t : t + 1],
                            in1=x_sb[t][:, sl],
                            op0=OP.add,
                            op1=OP.add,
                        )
                for t in range(CT):
                    for half in range(2):
                        ysl = slice(512 * half, 512 * half + 512)
                        (nc.sync if (t + half) % 2 else nc.scalar).dma_start(
                            yd[128 * t : 128 * t + 128, ysl], oh_sb[t][:, ysl]
                        )

    nc.compile()
    return nc


_NC_CACHE = None


def _get_nc():
    global _NC_CACHE
    if _NC_CACHE is None:
        _NC_CACHE = build_nc()
    return _NC_CACHE


def _host_inputs(x, gn_w, gn_b, w_qkv, w_out, b_out):
    w_qkvT = np.ascontiguousarray(w_qkv.T).astype(ml_dtypes.bfloat16)
    w_outT = np.ascontiguousarray(w_out.T).astype(ml_dtypes.bfloat16)
    ind = np.zeros((H, C), ml_dtypes.bfloat16)
    for h in range(H):
        for t in range(CT):
            for m in range(128):
                if h == 2 * t + (m >= 64):
                    ind[h, 128 * t + m] = 1.0
    ident = np.eye(128, dtype=np.float32)
    shared = {
        "wqkvT": w_qkvT,
        "woutT": w_outT,
        "gnw": np.ascontiguousarray(gn_w.reshape(CT, 128), np.float32),
        "gnb": np.ascontiguousarray(gn_b.reshape(CT, 128), np.float32),
        "bout": np.ascontiguousarray(b_out.reshape(CT, 128).T, np.float32),
        "ind": ind,
        "ident": ident,
    }
    in_maps = []
    for core in range(8):
        b, ih = core // 2, core % 2
        xb = np.asarray(x[b], np.float32)
        if ih:
            xb = np.concatenate([xb[:, LQ:], xb[:, :LQ]], axis=1)
        in_maps.append({"x": np.ascontiguousarray(xb), **shared})
    return in_maps


def kernel(x, gn_w, gn_b, w_qkv, w_out, b_out):
    nc = _get_nc()
    in_maps = _host_inputs(
        np.asarray(x), np.asarray(gn_w), np.asarray(gn_b),
        np.asarray(w_qkv), np.asarray(w_out), np.asarray(b_out),
    )
    res = run_bass_kernel_spmd(nc, in_maps, list(range(8)))
    y = np.empty((B, C, L), np.float32)
    for core in range(8):
        b, ih = core // 2, core % 2
        y[b][:, ih * LQ : (ih + 1) * LQ] = res.results[core]["y"]
    return y
